# revision 1
# baseline (speedup 1.0000x reference)
"""GAT message-passing kernel for Trainium2 (8 NeuronCores, batch data-parallel).

out[b,i,:] = sum_j softmax_j(mask(leaky_relu(el_i + er_j))) * h[b,j,:] + x[b,i,:]
  h = x @ W, el = x @ (W a_l), er = x @ (W a_r)
  mask: ADJ_BASE*adj_mask + I > 0.1

Layout: rows (b,n) flattened; tiles of 120 rows = 10 graphs; 8 tiles form one
"super-tile" for the attention elementwise chain ([120, 96] ops).
"""

import numpy as np
import ml_dtypes
from contextlib import ExitStack

import concourse.bass as bass
import concourse.bacc as bacc
import concourse.tile as tile
from concourse import mybir
from concourse.ap import AP
from concourse.bass_utils import run_bass_kernel_spmd
from concourse.bass_test_utils import get_trn_type

N = 12
C = 512
KC = C // 128            # 4 contraction chunks
NEG_SLOPE = 0.2
THRED = 0.1
N_CORES = 8
TILE_R = 120             # rows per matmul tile (10 graphs)
G_PER_TILE = TILE_R // N
ST_TILES = 8             # tiles per super-tile
BF16 = mybir.dt.bfloat16
F32 = mybir.dt.float32
NPBF16 = ml_dtypes.bfloat16

ADJ_BASE = np.array([
    [0,0,0,1,0,1,1,1,1,1,1,1],
    [0,0,0,1,0,1,1,1,1,1,1,1],
    [0,0,0,1,0,1,1,1,1,1,1,1],
    [1,1,1,0,1,1,1,1,1,1,1,1],
    [0,0,0,1,0,1,1,1,1,1,1,1],
    [1,1,1,1,1,0,1,1,1,0,0,0],
    [1,1,1,1,1,1,0,0,0,1,1,1],
    [1,1,1,1,1,1,0,0,0,1,1,1],
    [1,1,1,1,1,1,0,0,0,1,1,1],
    [1,1,1,1,1,0,1,1,1,0,0,0],
    [1,1,1,1,1,0,1,1,1,0,0,0],
    [1,1,1,1,1,0,1,1,1,0,0,0]], dtype=np.float32)


def host_consts():
    bo = np.kron(np.eye(G_PER_TILE, dtype=np.float32),
                 np.ones((N, N), dtype=np.float32))           # [120,120]
    tid = np.tile(np.eye(N, dtype=np.float32), (G_PER_TILE, 1))   # [120,12]
    adjb = np.tile(ADJ_BASE, (G_PER_TILE, ST_TILES))              # [120,96]
    idm = np.tile(np.eye(N, dtype=np.float32), (G_PER_TILE, ST_TILES))  # [120,96]
    i120 = np.eye(TILE_R, dtype=np.float32)                       # [120,120]
    return {
        "bo": bo.astype(NPBF16),
        "tid": tid.astype(NPBF16),
        "adjb": adjb.astype(np.float32),
        "idm": idm.astype(np.float32),
        "i120": i120.astype(NPBF16),
    }


def build_nc(n_tiles: int):
    """Build the per-core Bass program for n_tiles tiles of TILE_R rows."""
    rows = n_tiles * TILE_R
    rows_x = rows + 8        # transpose loads read [row0, row0+128)
    nc = bacc.Bacc(get_trn_type() or "TRN2", target_bir_lowering=False)
    nc.detect_race_conditions = False

    x_d = nc.declare_dram_parameter("x_bf", [rows_x, C], BF16, False)
    am_d = nc.declare_dram_parameter("adj", [rows, N], F32, False)
    w_d = nc.declare_dram_parameter("w_bf", [C, C], BF16, False)
    wlr_d = nc.declare_dram_parameter("wlr_bf", [C, 2], BF16, False)
    bo_d = nc.declare_dram_parameter("bo", [TILE_R, TILE_R], BF16, False)
    tid_d = nc.declare_dram_parameter("tid", [TILE_R, N], BF16, False)
    adjb_d = nc.declare_dram_parameter("adjb", [TILE_R, N * ST_TILES], F32, False)
    idm_d = nc.declare_dram_parameter("idm", [TILE_R, N * ST_TILES], F32, False)
    i120_d = nc.declare_dram_parameter("i120", [TILE_R, TILE_R], BF16, False)
    out_d = nc.declare_dram_parameter("out", [rows, C], F32, True)

    with ExitStack() as ctx:
        tc = ctx.enter_context(tile.TileContext(nc))
        _body(ctx, tc, n_tiles, x_d, am_d, w_d, wlr_d,
              bo_d, tid_d, adjb_d, idm_d, i120_d, out_d)
    nc.compile()
    return nc


def _body(ctx, tc, n_tiles, x_d, am_d, w_d, wlr_d,
          bo_d, tid_d, adjb_d, idm_d, i120_d, out_d):
    nc = tc.nc
    JW = N * ST_TILES   # 96

    cpool = ctx.enter_context(tc.tile_pool(name="consts", bufs=1))
    # resident weights / constants
    w_sb = cpool.tile([128, KC * C], BF16, name="w_sb")
    wlr_sb = cpool.tile([128, KC * 2], BF16, name="wlr_sb")
    for k in range(KC):
        nc.sync.dma_start(w_sb[:, k * C:(k + 1) * C], w_d[128 * k:128 * (k + 1), :])
        nc.sync.dma_start(wlr_sb[:, 2 * k:2 * k + 2], wlr_d[128 * k:128 * (k + 1), :])
    bo_sb = cpool.tile([TILE_R, TILE_R], BF16, name="bo_sb")
    nc.sync.dma_start(bo_sb[:], bo_d[:])
    tid_sb = cpool.tile([TILE_R, N], BF16, name="tid_sb")
    nc.sync.dma_start(tid_sb[:], tid_d[:])
    adjb_sb = cpool.tile([TILE_R, JW], F32, name="adjb_sb")
    nc.sync.dma_start(adjb_sb[:], adjb_d[:])
    idm_sb = cpool.tile([TILE_R, JW], F32, name="idm_sb")
    nc.sync.dma_start(idm_sb[:], idm_d[:])
    i120_sb = cpool.tile([TILE_R, TILE_R], BF16, name="i120_sb")
    nc.sync.dma_start(i120_sb[:], i120_d[:])

    # persistent block-diagonal alpha tiles (off-diagonal zeros written once)
    NBD = 3
    bd_tiles = []
    for bi in range(NBD):
        bdt = cpool.tile([TILE_R, TILE_R], BF16, name=f"bd{bi}_sb")
        nc.vector.memset(bdt[:], 0.0)
        bd_tiles.append(bdt)

    xn_pool = ctx.enter_context(tc.tile_pool(name="xn", bufs=12))
    xt_pool = ctx.enter_context(tc.tile_pool(name="xt", bufs=4))
    h_pool = ctx.enter_context(tc.tile_pool(name="h", bufs=12))
    o_pool = ctx.enter_context(tc.tile_pool(name="o", bufs=4))
    at_pool = ctx.enter_context(tc.tile_pool(name="attn", bufs=2))
    ph_pool = ctx.enter_context(tc.tile_pool(name="ph", bufs=2, space="PSUM"))
    pg_pool = ctx.enter_context(tc.tile_pool(name="pg", bufs=2, space="PSUM"))
    pe_pool = ctx.enter_context(tc.tile_pool(name="pe", bufs=2, space="PSUM"))
    pb_pool = ctx.enter_context(tc.tile_pool(name="pb", bufs=1, space="PSUM"))
    pt_pool = ctx.enter_context(tc.tile_pool(name="pt", bufs=1, space="PSUM"))

    n_st = (n_tiles + ST_TILES - 1) // ST_TILES
    bd_i = 0
    for st in range(n_st):
        t0 = st * ST_TILES
        nt = min(ST_TILES, n_tiles - t0)
        jw = N * nt

        # adjacency rows for the whole super-tile: [120, nt, 12]
        am_sup = at_pool.tile([TILE_R, JW], F32, tag="am")
        am_src = am_d[:].rearrange("(T p) j -> T p j", p=TILE_R)[t0:t0 + nt]
        nc.sync.dma_start(
            am_sup[:].rearrange("p (T j) -> p T j", j=N)[:, 0:nt],
            am_src.transpose([1, 0, 2]))

        elr_ps = pe_pool.tile([128, 2 * ST_TILES], F32, tag="elr")
        h_tiles = []
        xn_tiles = []
        for t in range(nt):
            row0 = (t0 + t) * TILE_R
            xn = xn_pool.tile([TILE_R, C], BF16, tag="xn")
            nc.sync.dma_start(xn[:], x_d[row0:row0 + TILE_R, :])
            xn_tiles.append(xn)

            xt = xt_pool.tile([128, KC * 128], BF16, tag="xt")
            for k in range(KC):
                nc.sync.dma_start(
                    out=xt[:, 128 * k:128 * (k + 1)],
                    in_=x_d[row0:row0 + 128, 128 * k:128 * (k + 1)],
                    transpose=True)

            ph = ph_pool.tile([128, C], F32, tag="ph")
            for k in range(KC):
                lhsT = xt[:, 128 * k:128 * (k + 1)]
                nc.tensor.matmul(ph[:], lhsT, w_sb[:, k * C:(k + 1) * C],
                                 start=(k == 0), stop=(k == KC - 1))
                nc.tensor.matmul(elr_ps[:, 2 * t:2 * t + 2], lhsT,
                                 wlr_sb[:, 2 * k:2 * k + 2],
                                 start=(k == 0), stop=(k == KC - 1))
            h_sb = h_pool.tile([TILE_R, C], BF16, tag="h")
            nc.scalar.copy(h_sb[:], ph[0:TILE_R, :])
            h_tiles.append(h_sb)

        # --- attention chain on [120, nt*12] ---
        # rhs_tid[p=(g,j'), (t,j)] = er_t[(g,j')] * (j'==j)
        rhs_tid = at_pool.tile([TILE_R, JW], BF16, tag="rhs_tid")
        tid3 = tid_sb[:].unsqueeze(1).broadcast_to([TILE_R, nt, N])
        er3 = elr_ps[0:TILE_R, 1:2 * nt:2].unsqueeze(2).broadcast_to([TILE_R, nt, N])
        nc.vector.tensor_tensor(
            rhs_tid[:].rearrange("p (T j) -> p T j", j=N)[:, 0:nt],
            tid3, er3, mybir.AluOpType.mult)

        # er_bcast[p=(g,i), (t,j)] = er_t[(g,j)]  via block-ones matmul
        eb_ps = pb_pool.tile([TILE_R, JW], F32, tag="eb")
        nc.tensor.matmul(eb_ps[:, 0:jw], bo_sb[:], rhs_tid[:, 0:jw],
                         start=True, stop=True)

        el8 = at_pool.tile([TILE_R, ST_TILES], F32, tag="el8")
        nc.vector.tensor_copy(el8[:, 0:nt], elr_ps[0:TILE_R, 0:2 * nt:2])

        # e = el + er_bcast ; e2 = lrelu(e)
        e_sb = at_pool.tile([TILE_R, JW], F32, tag="e_sb")
        el3 = el8[:, 0:nt].unsqueeze(2).broadcast_to([TILE_R, nt, N])
        nc.vector.tensor_tensor(
            e_sb[:].rearrange("p (T j) -> p T j", j=N)[:, 0:nt],
            eb_ps[:, 0:jw].rearrange("p (T j) -> p T j", j=N),
            el3, mybir.AluOpType.add)
        e2 = at_pool.tile([TILE_R, JW], F32, tag="e2")
        nc.vector.scalar_tensor_tensor(
            e2[:, 0:jw], e_sb[:, 0:jw], NEG_SLOPE, e_sb[:, 0:jw],
            mybir.AluOpType.mult, mybir.AluOpType.max)

        # pass = (adj_mask > 0.1)*ADJ_BASE + I
        q = at_pool.tile([TILE_R, JW], F32, tag="q")
        nc.vector.scalar_tensor_tensor(
            q[:, 0:jw], am_sup[:, 0:jw], THRED, adjb_sb[:, 0:jw],
            mybir.AluOpType.is_gt, mybir.AluOpType.mult)
        pass_ = at_pool.tile([TILE_R, JW], F32, tag="pass")
        nc.vector.tensor_tensor(pass_[:, 0:jw], q[:, 0:jw], idm_sb[:, 0:jw],
                                mybir.AluOpType.add)

        expv = at_pool.tile([TILE_R, JW], F32, tag="expv")
        nc.scalar.activation(expv[:, 0:jw], e2[:, 0:jw],
                             mybir.ActivationFunctionType.Exp)

        alphau = at_pool.tile([TILE_R, JW], BF16, tag="alphau")
        nc.vector.tensor_tensor(alphau[:, 0:jw], expv[:, 0:jw], pass_[:, 0:jw],
                                mybir.AluOpType.mult)

        s8 = at_pool.tile([TILE_R, ST_TILES], F32, tag="s8")
        nc.vector.tensor_reduce(
            s8[:, 0:nt],
            alphau[:].rearrange("p (T j) -> p T j", j=N)[:, 0:nt],
            mybir.AxisListType.X, mybir.AluOpType.add)
        recip8 = at_pool.tile([TILE_R, ST_TILES], F32, tag="recip8")
        nc.vector.reciprocal(recip8[:, 0:nt], s8[:, 0:nt])

        # transpose alpha: [120, nt*12] -> [nt*12, 120]
        paT = pt_pool.tile([JW, TILE_R], BF16, tag="paT")
        nc.tensor.matmul(paT[0:jw, :], alphau[:, 0:jw], i120_sb[:],
                         is_transpose=True)
        aT_sb = at_pool.tile([JW, TILE_R], BF16, tag="aT_sb")
        if nt < ST_TILES:
            nc.vector.memset(aT_sb[:], 0.0)
        nc.scalar.copy(aT_sb[0:jw, :], paT[0:jw, :])

        for t in range(nt):
            row0 = (t0 + t) * TILE_R
            # scatter alpha_t^T blocks onto the block diagonal of bd
            bd = bd_tiles[bd_i]
            bd_ap = bd[:]
            for g in range(G_PER_TILE):
                nc.gpsimd.dma_start(
                    out=bd[g * N:(g + 1) * N, g * N:(g + 1) * N],
                    in_=aT_sb[N * t:N * (t + 1), g * N:(g + 1) * N])

            pagg = pg_pool.tile([TILE_R, C], F32, tag="pagg")
            nc.tensor.matmul(pagg[:], bd_ap, h_tiles[t][:], start=True, stop=True)

            out_sb = o_pool.tile([TILE_R, C], F32, tag="out_sb")
            nc.vector.scalar_tensor_tensor(
                out_sb[:], pagg[:], recip8[:, t:t + 1], xn_tiles[t][:],
                mybir.AluOpType.mult, mybir.AluOpType.add)
            nc.sync.dma_start(out_d[row0:row0 + TILE_R, :], out_sb[:])
            bd_i = (bd_i + 1) % NBD


_NC_CACHE = {}


def _get_nc(n_tiles):
    if n_tiles not in _NC_CACHE:
        _NC_CACHE[n_tiles] = build_nc(n_tiles)
    return _NC_CACHE[n_tiles]


def prep_core_inputs(x, adj_mask, W, a_l, a_r):
    """Host-side prep: cast, pad, shard. Returns (in_maps, rows_real)."""
    B = x.shape[0]
    assert B % N_CORES == 0
    bpc = B // N_CORES
    rows_real = bpc * N
    n_tiles = (rows_real + TILE_R - 1) // TILE_R
    rows = n_tiles * TILE_R
    rows_x = rows + 8

    Wf = np.asarray(W, dtype=np.float32)
    wl = Wf @ np.asarray(a_l, dtype=np.float32)
    wr = Wf @ np.asarray(a_r, dtype=np.float32)
    w_bf = Wf.astype(NPBF16)
    wlr_bf = np.stack([wl, wr], axis=1).astype(NPBF16)
    consts = host_consts()

    x_bf_full = np.asarray(x, dtype=np.float32).astype(NPBF16)
    adj_full = np.asarray(adj_mask, dtype=np.float32)

    in_maps = []
    for c in range(N_CORES):
        xs = x_bf_full[c * bpc:(c + 1) * bpc].reshape(rows_real, C)
        xp = np.zeros((rows_x, C), dtype=NPBF16)
        xp[:rows_real] = xs
        ams = adj_full[c * bpc:(c + 1) * bpc].reshape(rows_real, N)
        amp = np.zeros((rows, N), dtype=np.float32)
        amp[:rows_real] = ams
        in_maps.append({
            "x_bf": xp, "adj": amp, "w_bf": w_bf, "wlr_bf": wlr_bf,
            "bo": consts["bo"], "tid": consts["tid"], "adjb": consts["adjb"],
            "idm": consts["idm"], "i120": consts["i120"],
        })
    return in_maps, rows_real, n_tiles


def kernel(x, adj_mask, W, a_l, a_r):
    x = np.asarray(x)
    in_dtype = x.dtype
    B = x.shape[0]
    in_maps, rows_real, n_tiles = prep_core_inputs(x, adj_mask, W, a_l, a_r)
    nc = _get_nc(n_tiles)
    res = run_bass_kernel_spmd(nc, in_maps, list(range(N_CORES)))
    bpc = B // N_CORES
    outs = [np.asarray(res.results[c]["out"][:rows_real]).reshape(bpc, N, C)
            for c in range(N_CORES)]
    return np.concatenate(outs, axis=0).astype(in_dtype, copy=False)



# revision 2
# speedup vs baseline: 4948.2695x; 4948.2695x over previous
"""GAT message-passing kernel for Trainium2 (8 NeuronCores, batch data-parallel).

out[b,i,:] = sum_j softmax_j(mask(leaky_relu(el_i + er_j))) * h[b,j,:] + x[b,i,:]
  h = x @ W, el = x @ (W a_l), er = x @ (W a_r)
  mask: ADJ_BASE*adj_mask + I > 0.1

Layout: rows (b,n) flattened; tiles of 120 rows = 10 graphs; 8 tiles form one
"super-tile" for the attention elementwise chain ([120, 96] ops).

v1 changes vs baseline:
  - x is read from HBM once per tile ([128,512] bf16); the transposed copy
    for the matmul lhsT is produced on-chip via DMA-XBAR SBUF->SBUF transpose.
  - output is stored bf16 (host upcasts); halves store traffic.
  - alpha transpose done on the DMA XBAR instead of the PE.
  - residual+scale op split across DVE and Pool(gpsimd) engines.
"""

import numpy as np
import ml_dtypes
from contextlib import ExitStack

import concourse.bass as bass
import concourse.bacc as bacc
import concourse.tile as tile
from concourse import mybir
from concourse.ap import AP
from concourse.bass_utils import run_bass_kernel_spmd
from concourse.bass_test_utils import get_trn_type

N = 12
C = 512
KC = C // 128            # 4 contraction chunks
NEG_SLOPE = 0.2
THRED = 0.1
N_CORES = 8
TILE_R = 120             # rows per matmul tile (10 graphs)
G_PER_TILE = TILE_R // N
ST_TILES = 8             # tiles per super-tile
BF16 = mybir.dt.bfloat16
F32 = mybir.dt.float32
NPBF16 = ml_dtypes.bfloat16

ADJ_BASE = np.array([
    [0,0,0,1,0,1,1,1,1,1,1,1],
    [0,0,0,1,0,1,1,1,1,1,1,1],
    [0,0,0,1,0,1,1,1,1,1,1,1],
    [1,1,1,0,1,1,1,1,1,1,1,1],
    [0,0,0,1,0,1,1,1,1,1,1,1],
    [1,1,1,1,1,0,1,1,1,0,0,0],
    [1,1,1,1,1,1,0,0,0,1,1,1],
    [1,1,1,1,1,1,0,0,0,1,1,1],
    [1,1,1,1,1,1,0,0,0,1,1,1],
    [1,1,1,1,1,0,1,1,1,0,0,0],
    [1,1,1,1,1,0,1,1,1,0,0,0],
    [1,1,1,1,1,0,1,1,1,0,0,0]], dtype=np.float32)


def host_consts():
    bo = np.kron(np.eye(G_PER_TILE, dtype=np.float32),
                 np.ones((N, N), dtype=np.float32))           # [120,120]
    tid = np.tile(np.eye(N, dtype=np.float32), (G_PER_TILE, 1))   # [120,12]
    adjb = np.tile(ADJ_BASE, (G_PER_TILE, ST_TILES))              # [120,96]
    idm = np.tile(np.eye(N, dtype=np.float32), (G_PER_TILE, ST_TILES))  # [120,96]
    return {
        "bo": bo.astype(NPBF16),
        "tid": tid.astype(NPBF16),
        "adjb": adjb.astype(np.float32),
        "idm": idm.astype(np.float32),
    }


def build_nc(n_tiles: int):
    """Build the per-core Bass program for n_tiles tiles of TILE_R rows."""
    rows = n_tiles * TILE_R
    rows_x = rows + 8        # tile loads read [row0, row0+128)
    nc = bacc.Bacc(get_trn_type() or "TRN2", target_bir_lowering=False)
    nc.detect_race_conditions = False

    x_d = nc.declare_dram_parameter("x_bf", [rows_x, C], BF16, False)
    am_d = nc.declare_dram_parameter("adj", [rows, N], F32, False)
    w_d = nc.declare_dram_parameter("w_bf", [C, C], BF16, False)
    wlr_d = nc.declare_dram_parameter("wlr_bf", [C, 2], BF16, False)
    bo_d = nc.declare_dram_parameter("bo", [TILE_R, TILE_R], BF16, False)
    tid_d = nc.declare_dram_parameter("tid", [TILE_R, N], BF16, False)
    adjb_d = nc.declare_dram_parameter("adjb", [TILE_R, N * ST_TILES], F32, False)
    idm_d = nc.declare_dram_parameter("idm", [TILE_R, N * ST_TILES], F32, False)
    out_d = nc.declare_dram_parameter("out", [rows, C], BF16, True)

    with ExitStack() as ctx:
        tc = ctx.enter_context(tile.TileContext(nc))
        _body(ctx, tc, n_tiles, x_d, am_d, w_d, wlr_d,
              bo_d, tid_d, adjb_d, idm_d, out_d)
    nc.compile()
    return nc


def _body(ctx, tc, n_tiles, x_d, am_d, w_d, wlr_d,
          bo_d, tid_d, adjb_d, idm_d, out_d):
    nc = tc.nc
    JW = N * ST_TILES   # 96

    cpool = ctx.enter_context(tc.tile_pool(name="consts", bufs=1))
    # resident weights / constants
    w_sb = cpool.tile([128, KC * C], BF16, name="w_sb")
    wlr_sb = cpool.tile([128, KC * 2], BF16, name="wlr_sb")
    for k in range(KC):
        nc.sync.dma_start(w_sb[:, k * C:(k + 1) * C], w_d[128 * k:128 * (k + 1), :])
        nc.sync.dma_start(wlr_sb[:, 2 * k:2 * k + 2], wlr_d[128 * k:128 * (k + 1), :])
    bo_sb = cpool.tile([TILE_R, TILE_R], BF16, name="bo_sb")
    nc.sync.dma_start(bo_sb[:], bo_d[:])
    tid_sb = cpool.tile([TILE_R, N], BF16, name="tid_sb")
    nc.sync.dma_start(tid_sb[:], tid_d[:])
    adjb_sb = cpool.tile([TILE_R, JW], F32, name="adjb_sb")
    nc.sync.dma_start(adjb_sb[:], adjb_d[:])
    idm_sb = cpool.tile([TILE_R, JW], F32, name="idm_sb")
    nc.sync.dma_start(idm_sb[:], idm_d[:])

    # persistent block-diagonal alpha tiles (off-diagonal zeros written once)
    NBD = 3
    bd_tiles = []
    for bi in range(NBD):
        bdt = cpool.tile([TILE_R, TILE_R], BF16, name=f"bd{bi}_sb")
        nc.vector.memset(bdt[:], 0.0)
        bd_tiles.append(bdt)

    xn_pool = ctx.enter_context(tc.tile_pool(name="xn", bufs=12))
    xt_pool = ctx.enter_context(tc.tile_pool(name="xt", bufs=4))
    h_pool = ctx.enter_context(tc.tile_pool(name="h", bufs=12))
    o_pool = ctx.enter_context(tc.tile_pool(name="o", bufs=4))
    at_pool = ctx.enter_context(tc.tile_pool(name="attn", bufs=2))
    ph_pool = ctx.enter_context(tc.tile_pool(name="ph", bufs=2, space="PSUM"))
    pg_pool = ctx.enter_context(tc.tile_pool(name="pg", bufs=2, space="PSUM"))
    pe_pool = ctx.enter_context(tc.tile_pool(name="pe", bufs=2, space="PSUM"))
    pb_pool = ctx.enter_context(tc.tile_pool(name="pb", bufs=1, space="PSUM"))

    OS = 288             # DVE handles out[:, :OS]; Pool handles the rest

    n_st = (n_tiles + ST_TILES - 1) // ST_TILES
    bd_i = 0
    for st in range(n_st):
        t0 = st * ST_TILES
        nt = min(ST_TILES, n_tiles - t0)
        jw = N * nt

        # adjacency rows for the whole super-tile: [120, nt, 12]
        am_sup = at_pool.tile([TILE_R, JW], F32, tag="am")
        am_src = am_d[:].rearrange("(T p) j -> T p j", p=TILE_R)[t0:t0 + nt]
        nc.sync.dma_start(
            am_sup[:].rearrange("p (T j) -> p T j", j=N)[:, 0:nt],
            am_src.transpose([1, 0, 2]))

        elr_ps = pe_pool.tile([128, 2 * ST_TILES], F32, tag="elr")
        h_tiles = []
        xn_tiles = []
        for t in range(nt):
            row0 = (t0 + t) * TILE_R
            xn = xn_pool.tile([128, C], BF16, tag="xn")
            nc.sync.dma_start(xn[:], x_d[row0:row0 + 128, :])
            xn_tiles.append(xn)

            # on-chip transpose: [128 rows, 512 c] -> 4 x [128 c-chunk, 128 rows]
            xt = xt_pool.tile([128, KC * 128], BF16, tag="xt")
            for k in range(KC):
                nc.scalar.dma_start(
                    out=xt[:, 128 * k:128 * (k + 1)],
                    in_=xn[:, 128 * k:128 * (k + 1)],
                    transpose=True)

            ph = ph_pool.tile([128, C], F32, tag="ph")
            for k in range(KC):
                lhsT = xt[:, 128 * k:128 * (k + 1)]
                nc.tensor.matmul(ph[:], lhsT, w_sb[:, k * C:(k + 1) * C],
                                 start=(k == 0), stop=(k == KC - 1))
                nc.tensor.matmul(elr_ps[:, 2 * t:2 * t + 2], lhsT,
                                 wlr_sb[:, 2 * k:2 * k + 2],
                                 start=(k == 0), stop=(k == KC - 1))
            h_sb = h_pool.tile([TILE_R, C], BF16, tag="h")
            nc.scalar.copy(h_sb[:], ph[0:TILE_R, :])
            h_tiles.append(h_sb)

        # --- attention chain on [120, nt*12] ---
        # rhs_tid[p=(g,j'), (t,j)] = er_t[(g,j')] * (j'==j)
        rhs_tid = at_pool.tile([TILE_R, JW], BF16, tag="rhs_tid")
        tid3 = tid_sb[:].unsqueeze(1).broadcast_to([TILE_R, nt, N])
        er3 = elr_ps[0:TILE_R, 1:2 * nt:2].unsqueeze(2).broadcast_to([TILE_R, nt, N])
        nc.vector.tensor_tensor(
            rhs_tid[:].rearrange("p (T j) -> p T j", j=N)[:, 0:nt],
            tid3, er3, mybir.AluOpType.mult)

        # er_bcast[p=(g,i), (t,j)] = er_t[(g,j)]  via block-ones matmul
        eb_ps = pb_pool.tile([TILE_R, JW], F32, tag="eb")
        nc.tensor.matmul(eb_ps[:, 0:jw], bo_sb[:], rhs_tid[:, 0:jw],
                         start=True, stop=True)

        el8 = at_pool.tile([TILE_R, ST_TILES], F32, tag="el8")
        nc.vector.tensor_copy(el8[:, 0:nt], elr_ps[0:TILE_R, 0:2 * nt:2])

        # e = el + er_bcast ; e2 = lrelu(e)
        e_sb = at_pool.tile([TILE_R, JW], F32, tag="e_sb")
        el3 = el8[:, 0:nt].unsqueeze(2).broadcast_to([TILE_R, nt, N])
        nc.vector.tensor_tensor(
            e_sb[:].rearrange("p (T j) -> p T j", j=N)[:, 0:nt],
            eb_ps[:, 0:jw].rearrange("p (T j) -> p T j", j=N),
            el3, mybir.AluOpType.add)
        e2 = at_pool.tile([TILE_R, JW], F32, tag="e2")
        nc.vector.scalar_tensor_tensor(
            e2[:, 0:jw], e_sb[:, 0:jw], NEG_SLOPE, e_sb[:, 0:jw],
            mybir.AluOpType.mult, mybir.AluOpType.max)

        # pass = (adj_mask > 0.1)*ADJ_BASE + I
        q = at_pool.tile([TILE_R, JW], F32, tag="q")
        nc.vector.scalar_tensor_tensor(
            q[:, 0:jw], am_sup[:, 0:jw], THRED, adjb_sb[:, 0:jw],
            mybir.AluOpType.is_gt, mybir.AluOpType.mult)
        pass_ = at_pool.tile([TILE_R, JW], F32, tag="pass")
        nc.vector.tensor_tensor(pass_[:, 0:jw], q[:, 0:jw], idm_sb[:, 0:jw],
                                mybir.AluOpType.add)

        expv = at_pool.tile([TILE_R, JW], F32, tag="expv")
        nc.scalar.activation(expv[:, 0:jw], e2[:, 0:jw],
                             mybir.ActivationFunctionType.Exp)

        # alphau padded to [128,128] so the DMA XBAR can transpose it
        alphau = at_pool.tile([128, 128], BF16, tag="alphau")
        nc.vector.tensor_tensor(alphau[0:TILE_R, 0:jw], expv[:, 0:jw],
                                pass_[:, 0:jw], mybir.AluOpType.mult)

        s8 = at_pool.tile([TILE_R, ST_TILES], F32, tag="s8")
        nc.vector.tensor_reduce(
            s8[:, 0:nt],
            alphau[0:TILE_R, 0:JW].rearrange("p (T j) -> p T j", j=N)[:, 0:nt],
            mybir.AluOpType.add)
        recip8 = at_pool.tile([TILE_R, ST_TILES], F32, tag="recip8")
        nc.vector.reciprocal(recip8[:, 0:nt], s8[:, 0:nt])

        # transpose alpha on the DMA XBAR: [120,jw] -> [jw,120] (inside 128x128)
        aT_sb = at_pool.tile([128, 128], BF16, tag="aT_sb")
        nc.scalar.dma_start(out=aT_sb[:], in_=alphau[:], transpose=True)

        for t in range(nt):
            row0 = (t0 + t) * TILE_R
            # scatter alpha_t^T blocks onto the block diagonal of bd
            bd = bd_tiles[bd_i]
            bd_ap = bd[:]
            for g in range(G_PER_TILE):
                nc.gpsimd.dma_start(
                    out=bd[g * N:(g + 1) * N, g * N:(g + 1) * N],
                    in_=aT_sb[N * t:N * (t + 1), g * N:(g + 1) * N])

            pagg = pg_pool.tile([TILE_R, C], F32, tag="pagg")
            nc.tensor.matmul(pagg[:], bd_ap, h_tiles[t][:], start=True, stop=True)

            out_sb = o_pool.tile([TILE_R, C], BF16, tag="out_sb")
            nc.vector.scalar_tensor_tensor(
                out_sb[:, 0:OS], pagg[:, 0:OS], recip8[:, t:t + 1],
                xn_tiles[t][0:TILE_R, 0:OS],
                mybir.AluOpType.mult, mybir.AluOpType.add)
            nc.gpsimd.scalar_tensor_tensor(
                out_sb[:, OS:C], pagg[:, OS:C], recip8[:, t:t + 1],
                xn_tiles[t][0:TILE_R, OS:C],
                mybir.AluOpType.mult, mybir.AluOpType.add)
            nc.sync.dma_start(out_d[row0:row0 + TILE_R, :], out_sb[:])
            bd_i = (bd_i + 1) % NBD


_NC_CACHE = {}


def _get_nc(n_tiles):
    if n_tiles not in _NC_CACHE:
        _NC_CACHE[n_tiles] = build_nc(n_tiles)
    return _NC_CACHE[n_tiles]


def prep_core_inputs(x, adj_mask, W, a_l, a_r):
    """Host-side prep: cast, pad, shard. Returns (in_maps, rows_real)."""
    B = x.shape[0]
    assert B % N_CORES == 0
    bpc = B // N_CORES
    rows_real = bpc * N
    n_tiles = (rows_real + TILE_R - 1) // TILE_R
    rows = n_tiles * TILE_R
    rows_x = rows + 8

    Wf = np.asarray(W, dtype=np.float32)
    wl = Wf @ np.asarray(a_l, dtype=np.float32)
    wr = Wf @ np.asarray(a_r, dtype=np.float32)
    w_bf = Wf.astype(NPBF16)
    wlr_bf = np.stack([wl, wr], axis=1).astype(NPBF16)
    consts = host_consts()

    x_bf_full = np.asarray(x, dtype=np.float32).astype(NPBF16)
    adj_full = np.asarray(adj_mask, dtype=np.float32)

    in_maps = []
    for c in range(N_CORES):
        xs = x_bf_full[c * bpc:(c + 1) * bpc].reshape(rows_real, C)
        xp = np.zeros((rows_x, C), dtype=NPBF16)
        xp[:rows_real] = xs
        ams = adj_full[c * bpc:(c + 1) * bpc].reshape(rows_real, N)
        amp = np.zeros((rows, N), dtype=np.float32)
        amp[:rows_real] = ams
        in_maps.append({
            "x_bf": xp, "adj": amp, "w_bf": w_bf, "wlr_bf": wlr_bf,
            "bo": consts["bo"], "tid": consts["tid"], "adjb": consts["adjb"],
            "idm": consts["idm"],
        })
    return in_maps, rows_real, n_tiles


def kernel(x, adj_mask, W, a_l, a_r):
    x = np.asarray(x)
    in_dtype = x.dtype
    B = x.shape[0]
    in_maps, rows_real, n_tiles = prep_core_inputs(x, adj_mask, W, a_l, a_r)
    nc = _get_nc(n_tiles)
    res = run_bass_kernel_spmd(nc, in_maps, list(range(N_CORES)))
    bpc = B // N_CORES
    outs = [np.asarray(res.results[c]["out"][:rows_real])
            .astype(np.float32).reshape(bpc, N, C)
            for c in range(N_CORES)]
    return np.concatenate(outs, axis=0).astype(in_dtype, copy=False)
